# revision 22
# baseline (speedup 1.0000x reference)
"""Trainium2 Bass kernel for nn_EquivariancePermutationLayer.

Math restructuring (all MLPs have ZERO biases => positively homogeneous):
  * 1-D MLPs (f_Imij/f_Imii/f_Mmimj/f_Mmimi) collapse exactly to
        g(s) = g(1)*relu(s) + g(-1)*relu(-s)            (2 constants)
  * output = sum_{k=0..15} C[b,k] * x[b,k,:,:] with 16 per-element coeffs:
        C[:, 0:5]  = f_Mii(sii2_i, v4_i, rii, mii)       (direct MLP)
        C[:, 5:15] = f_Mij(sij2_p, v3_p, rij, mij)       (direct MLP)
        C[:, 15]   = sum_p f_Iij(sij2_p, v1_p) + sum_i f_Iii(sii2_i, v2_i)
  * globals rij/mij/rii/mii: direct MLPs on 15-D / 5-D inputs.
  * All per-(row) MLP layers run as TensorE matmuls with rows streamed on
    the free dim (hidden on partitions); the 16 coefficients accumulate
    into one [16, E] PSUM tile via column-packed final-layer stationaries.
  * optional 'refit' mode for the 2-D nets: exact homogenized PWL form
    F(u,v) = a+u+ + b+v+ + sum_k g+_k relu(v+ - t_k u+) + (mirror u<0),
    with least-squares-refit kink tables (approximation, ~2.5e-3 rel-l2).

Data parallel across 8 NeuronCores (16384 elements each), no collectives.
"""

import numpy as np

NCORES = 8
H = 128

# ---- tuning knobs ------------------------------------------------------
FP32R = True          # use float32r (fast reduced-precision) matmuls
FI_MODE = "direct"    # "direct" (exact) or "refit" (approx, fewer PE cols)
REFIT_K = 61          # kinks per half-plane in refit mode (61+61+6 = 128)
E_CHUNK = 512         # elements per chunk (multiple of 128; PSUM bank = 512 fp32)

_COMPILE_CACHE = {}


# ======================================================================
# host-side math helpers
# ======================================================================

def _np_params(params):
    return {k: [(np.asarray(W, np.float64), np.asarray(b, np.float64))
                for W, b in v] for k, v in params.items()}


def _mlp64(layers, h):
    for W, b in layers[:-1]:
        h = np.maximum(h @ W + b, 0)
    W, b = layers[-1]
    return h @ W + b


def _fit_homog_tables(layers, u, v, n_kinks):
    """Least-squares homogenized-PWL refit of a 2-input zero-bias MLP."""
    uf = u.ravel().astype(np.float64)
    vf = v.ravel().astype(np.float64)
    tabs = {}
    for side in (+1, -1):
        mask = (uf > 0) if side > 0 else (uf <= 0)
        us = np.abs(uf[mask])
        t = vf[mask] / np.maximum(us, 1e-300)
        w = us
        order = np.argsort(t)
        tw = t[order]
        cw = np.cumsum(w[order] ** 2)
        cw /= cw[-1]
        kinks = np.unique(np.interp(np.linspace(0.002, 0.998, n_kinks), cw, tw))
        ns = min(300_000, len(t))
        idx = np.random.default_rng(0).choice(len(t), ns, replace=False)
        ts, ws = t[idx], w[idx]
        phi = _mlp64(layers, np.stack([np.full_like(ts, side), ts], -1))[:, 0]
        A = np.concatenate(
            [np.ones((ns, 1)), ts[:, None], np.maximum(ts[:, None] - kinks[None, :], 0)], 1)
        coef, *_ = np.linalg.lstsq(A * ws[:, None], phi * ws, rcond=None)
        # pad kink count up to n_kinks (duplicates with zero gamma)
        kk = np.zeros(n_kinks); gg = np.zeros(n_kinks)
        kk[:len(kinks)] = kinks
        kk[len(kinks):] = kinks[-1] if len(kinks) else 0.0
        gg[:len(kinks)] = coef[2:]
        tabs[side] = (np.float64(coef[0]), np.float64(coef[1]), kk, gg)
    return tabs


# ======================================================================
# constants blob layout
# ======================================================================

class CstPacker:
    def __init__(self):
        self.chunks = []
        self.offsets = {}
        self.size = 0

    def add(self, name, arr):
        arr = np.ascontiguousarray(np.asarray(arr, np.float32))
        self.offsets[name] = (self.size, arr.shape)
        self.chunks.append(arr.ravel())
        self.size += arr.size

    def blob(self):
        return np.concatenate(self.chunks).astype(np.float32)


def _build_constants(params, scalars=None):
    """Pack all device constants. scalars (full array) only needed for refit."""
    P = _np_params(params)
    cst = CstPacker()

    # g-collapse constants
    g = {}
    for n in ["f_Imij", "f_Imii", "f_Mmimj", "f_Mmimi"]:
        cp = _mlp64(P[n], np.array([[1.0]]))[0, 0]
        cn = _mlp64(P[n], np.array([[-1.0]]))[0, 0]
        g[n] = (np.float32(cp), np.float32(cn))

    def addW(name, key, li):
        cst.add(name, P[key][li][0])

    # global nets (full weights)
    for key, tag in [("f_Mrij", "rij"), ("f_Mmij", "mij"),
                     ("f_Mrii", "rii"), ("f_Mmii", "mii")]:
        addW(f"W1_{tag}", key, 0)
        addW(f"W2_{tag}", key, 1)
        addW(f"W3_{tag}", key, 2)
    # packed final layer of globals -> RM psum rows (rij, mij, rii, mii)
    for j, key in enumerate(["f_Mrij", "f_Mmij", "f_Mrii", "f_Mmii"]):
        w4 = np.zeros((128, 4), np.float32)
        w4[:, j] = P[key][3][0][:, 0]
        cst.add(f"W4RM_{j}", w4)

    # F_M nets
    for key, tag in [("f_Mij", "Mij"), ("f_Mii", "Mii")]:
        addW(f"W1_{tag}", key, 0)     # [4,128]
        addW(f"W2_{tag}", key, 1)
        addW(f"W3_{tag}", key, 2)
    for p in range(10):
        w4 = np.zeros((128, 16), np.float32)
        w4[:, 5 + p] = P["f_Mij"][3][0][:, 0]
        cst.add(f"W4C_Mij_{p}", w4)
    for i in range(5):
        w4 = np.zeros((128, 16), np.float32)
        w4[:, i] = P["f_Mii"][3][0][:, 0]
        cst.add(f"W4C_Mii_{i}", w4)

    # F_I nets
    if FI_MODE == "direct":
        for key, tag in [("f_Iij", "Iij"), ("f_Iii", "Iii")]:
            addW(f"W1_{tag}", key, 0)   # [2,128]
            addW(f"W2_{tag}", key, 1)
            addW(f"W3_{tag}", key, 2)
            w4 = np.zeros((128, 16), np.float32)
            w4[:, 15] = P[key][3][0][:, 0]
            cst.add(f"W4C_{tag}", w4)
    else:
        assert scalars is not None
        sc = np.asarray(scalars, np.float32)
        sii = sc[:, :5]; sij = sc[:, 5:]
        s0, s1 = sij[..., 0], sij[..., 1]
        v1 = g["f_Imij"][0] * (np.maximum(s0, 0) + np.maximum(s1, 0)) \
           + g["f_Imij"][1] * (np.maximum(-s0, 0) + np.maximum(-s1, 0))
        d0 = sii[..., 0]
        v2 = g["f_Imii"][0] * np.maximum(d0, 0) + g["f_Imii"][1] * np.maximum(-d0, 0)
        for key, tag, uu, vv in [("f_Iij", "Iij", sij[..., 2], v1),
                                 ("f_Iii", "Iii", sii[..., 2], v2)]:
            tb = _fit_homog_tables(P[key], uu, vv, REFIT_K)
            # outer stationary [4, 128]: rows (u+, v+, u-, v-);
            # cols 0..K-1:   (-t+_k, 1, 0, 0)
            # cols K..2K-1:  (0, 0, -t-_k, 1)
            # cols 2K..2K+5: u+ | v+ | -v+ | u- | v- | -v-
            K = REFIT_K
            ap, bp, tp, gp = tb[+1]
            an, bn, tn, gn = tb[-1]
            W = np.zeros((4, 128), np.float32)
            W[0, :K] = -tp; W[1, :K] = 1.0
            W[2, K:2*K] = -tn; W[3, K:2*K] = 1.0
            W[0, 2*K] = 1.0
            W[1, 2*K+1] = 1.0; W[1, 2*K+2] = -1.0
            W[2, 2*K+3] = 1.0
            W[3, 2*K+4] = 1.0; W[3, 2*K+5] = -1.0
            cst.add(f"WO_{tag}", W)
            gam = np.zeros((128, 16), np.float32)
            gam[:K, 15] = gp
            gam[K:2*K, 15] = gn
            gam[2*K:2*K+6, 15] = [ap, bp, bp, an, bn, bn]
            gam[2*K+2, 15] *= -1.0   # -b+ relu(-v+)
            gam[2*K+5, 15] *= -1.0
            cst.add(f"GAM_{tag}", gam)

    cst.add("I16", np.eye(16, dtype=np.float32))
    cst.add("I128", np.eye(128, dtype=np.float32))
    return cst, g


# ======================================================================
# device kernel builder
# ======================================================================

def _build_nc(n_elem, cst_meta, g_const):
    import concourse.bass as bass
    import concourse.mybir as mybir
    from concourse.tile import TileContext
    from concourse.tile_rust import add_dep_helper
    from contextlib import ExitStack

    f32 = mybir.dt.float32
    f32r = mybir.dt.float32r
    Relu = mybir.ActivationFunctionType.Relu
    Alu = mybir.AluOpType

    E = E_CHUNK
    NJ = E // 128                     # e-groups per chunk
    NCH = n_elem // E
    assert n_elem % E == 0

    nc = bass.Bass()
    d_sc = nc.declare_dram_parameter("scalars", (n_elem, 45), f32, isOutput=False)
    d_x = nc.declare_dram_parameter("x", (n_elem, 144), f32, isOutput=False)
    d_cst = nc.declare_dram_parameter("cst", (cst_meta.size,), f32, isOutput=False)
    d_out = nc.declare_dram_parameter("out", (n_elem, 9), f32, isOutput=True)

    (c1p, c1n) = map(float, g_const["f_Imij"])
    (c2p, c2n) = map(float, g_const["f_Imii"])
    (c3p, c3n) = map(float, g_const["f_Mmimj"])
    (c4p, c4n) = map(float, g_const["f_Mmimi"])

    dtm = f32r if FP32R else f32

    def r_(ap):
        return ap

    with TileContext(nc) as tc, ExitStack() as ctx:
        consts = ctx.enter_context(tc.tile_pool(name="consts", bufs=1))
        iop = ctx.enter_context(tc.tile_pool(name="iop", bufs=3))
        elp = ctx.enter_context(tc.tile_pool(name="elp", bufs=2))
        stg = ctx.enter_context(tc.tile_pool(name="stg", bufs=2))
        hp = ctx.enter_context(tc.tile_pool(name="hp", bufs=6))
        outp = ctx.enter_context(tc.tile_pool(name="outp", bufs=3))
        zp = ctx.enter_context(tc.tile_pool(name="zp", bufs=4, space="PSUM"))
        cp = ctx.enter_context(tc.tile_pool(name="cp", bufs=2, space="PSUM"))

        def evict(out_ap, in_ap, relu=True, eng="v"):
            if eng == "v":
                if relu:
                    return nc.vector.tensor_scalar_max(out_ap, in_ap, 0.0)
                return nc.vector.tensor_copy(out_ap, in_ap)
            return nc.scalar.activation(
                out_ap, in_ap, Relu if relu else mybir.ActivationFunctionType.Copy)

        def _raw(i):
            return i.ins if hasattr(i, "ins") and not isinstance(i.ins, list) else i

        def mm(out, lhsT, rhs, start=True, stop=True, after=None):
            return nc.tensor.matmul(out, r_(lhsT), r_(rhs), start=start, stop=stop)

        def chain128(h1, tagW2, tagW3):
            """h1 [128,E] sbuf -> returns h3 [128,E] sbuf"""
            z2 = zp.tile([128, E], f32, tag="z")
            mm(z2, W[tagW2], h1)
            h2 = hp.tile([128, E], dtm, tag="h")
            evict(h2, z2, eng="a")
            z3 = zp.tile([128, E], f32, tag="z")
            mm(z3, W[tagW3], h2)
            h3 = hp.tile([128, E], dtm, tag="h")
            evict(h3, z3, eng="v")
            return h3

        # ---- load constants once ----
        W = {}
        for name, (off, shape) in cst_meta.offsets.items():
            dt_w = f32 if name in ("I16", "I128") else dtm
            t = consts.tile(list(shape), dt_w, tag=f"c_{name}")
            src = bass.AP(tensor=d_cst, offset=off,
                          ap=[[shape[-1], shape[0]], [1, shape[-1]]]
                          if len(shape) == 2 else [[1, shape[0]]])
            if dt_w is not f32:
                src = src.bitcast(dt_w)
            nc.sync.dma_start(out=t, in_=src)
            W[name] = t

        for ch in range(NCH):
            e0 = ch * E

            # ---------------- loads ----------------
            sraw = iop.tile([128, NJ, 45], f32, tag="sraw")
            nc.sync.dma_start(
                out=sraw,
                in_=bass.AP(tensor=d_sc, offset=e0 * 45,
                            ap=[[45, 128], [45 * 128, NJ], [1, 45]]))
            xt = iop.tile([128, NJ, 144], f32, tag="xt")
            nc.sync.dma_start(
                out=xt,
                in_=bass.AP(tensor=d_x, offset=e0 * 144,
                            ap=[[144, 128], [144 * 128, NJ], [1, 144]]))

            # ---------------- e-layout g computation ----------------
            # Q columns: 0..9 uA | 10..19 v3 | 20..29 uB | 30..39 v1
            #   40..44 udA | 45..49 v4 | 50..54 udB | 55..59 v2
            #   60..74 sbar | 75..79 sii0
            QC = 80
            Q = elp.tile([128, NJ, QC], f32, tag="Q")

            def sl(col0, stride, n):
                return sraw[:, :, col0:col0 + stride * (n - 1) + 1:stride]

            s0ap = sl(15, 3, 10); s1ap = sl(16, 3, 10); d0ap = sl(0, 3, 5)

            # raw input columns
            nc.vector.tensor_copy(Q[:, :, 0:10], sl(17, 3, 10))
            nc.vector.tensor_copy(Q[:, :, 20:30], sl(17, 3, 10))
            nc.vector.tensor_copy(Q[:, :, 40:45], sl(2, 3, 5))
            nc.vector.tensor_copy(Q[:, :, 50:55], sl(2, 3, 5))
            nc.vector.tensor_copy(Q[:, :, 60:75], sl(2, 3, 15))
            nc.vector.tensor_copy(Q[:, :, 75:80], sl(0, 3, 5))

            t0 = elp.tile([128, NJ, 10], f32, tag="t0")
            t1 = elp.tile([128, NJ, 10], f32, tag="t1")
            Sp = elp.tile([128, NJ, 10], f32, tag="Sp")
            nc.vector.tensor_scalar_max(t0, s0ap, 0.0)
            nc.vector.tensor_scalar_max(t1, s1ap, 0.0)
            nc.vector.tensor_tensor(Sp, t0, t1, Alu.add)
            t0n = elp.tile([128, NJ, 10], f32, tag="t0n")
            t1n = elp.tile([128, NJ, 10], f32, tag="t1n")
            Sn = elp.tile([128, NJ, 10], f32, tag="Sn")
            nc.vector.tensor_scalar(t0n, s0ap, -1.0, 0.0, Alu.mult, Alu.max)
            nc.vector.tensor_scalar(t1n, s1ap, -1.0, 0.0, Alu.mult, Alu.max)
            nc.vector.tensor_tensor(Sn, t0n, t1n, Alu.add)

            tmp = elp.tile([128, NJ, 10], f32, tag="tmpg")
            nc.vector.tensor_scalar_mul(tmp, Sp, c3p)
            nc.vector.scalar_tensor_tensor(Q[:, :, 10:20], Sn, c3n, tmp, Alu.mult, Alu.add)
            nc.vector.tensor_scalar_mul(tmp, Sp, c1p)
            nc.vector.scalar_tensor_tensor(Q[:, :, 30:40], Sn, c1n, tmp, Alu.mult, Alu.add)

            dp = elp.tile([128, NJ, 5], f32, tag="dp")
            dn = elp.tile([128, NJ, 5], f32, tag="dn")
            nc.vector.tensor_scalar_max(dp, d0ap, 0.0)
            nc.vector.tensor_scalar(dn, d0ap, -1.0, 0.0, Alu.mult, Alu.max)
            tmpd = elp.tile([128, NJ, 5], f32, tag="tmpd")
            nc.vector.tensor_scalar_mul(tmpd, dp, c4p)
            nc.vector.scalar_tensor_tensor(Q[:, :, 45:50], dn, c4n, tmpd, Alu.mult, Alu.add)
            nc.vector.tensor_scalar_mul(tmpd, dp, c2p)
            nc.vector.scalar_tensor_tensor(Q[:, :, 55:60], dn, c2n, tmpd, Alu.mult, Alu.add)

            # ---------------- transpose Q -> QT (features on partitions) ----
            QT = elp.tile([QC, NJ, 128], dtm, tag="QT")
            for j in range(NJ):
                zt = zp.tile([128, E], f32, tag="z", name="zt")
                nc.tensor.transpose(zt[0:QC, 0:128], Q[:, j, :], W["I128"])
                evict(QT[:, j, :], zt[0:QC, 0:128], relu=False, eng="v")

            # ---------------- globals (PE) ----------------
            def qt_rows(a, b):
                return QT[a:b].rearrange("q j i -> q (j i)")

            Tg15 = stg.tile([15, E], dtm, tag="Tg15")
            nc.sync.dma_start(out=Tg15, in_=qt_rows(60, 75))
            Tg5 = stg.tile([5, E], dtm, tag="Tg5")
            nc.sync.dma_start(out=Tg5, in_=qt_rows(75, 80))

            RM = cp.tile([4, E], f32, tag="rm_psum")
            for j, tag in enumerate(["rij", "mij", "rii", "mii"]):
                rhs_g = Tg15 if tag in ("rij", "rii") else Tg5
                z1 = zp.tile([128, E], f32, tag="z", name="z1g")
                mm(z1, W[f"W1_{tag}"], rhs_g)
                h1 = hp.tile([128, E], dtm, tag="h", name="h1g")
                evict(h1, z1)
                h3 = chain128(h1, f"W2_{tag}", f"W3_{tag}")
                mm(RM, W[f"W4RM_{j}"], h3, start=(j == 0), stop=(j == 3))
            rm4 = outp.tile([4, E], dtm, tag="rm4")
            nc.vector.tensor_copy(rm4, RM)
            rm_ij = rm4[0:2]
            rm_ii = rm4[2:4]

            # ---------------- staging tiles [K, P, E] ----------------
            TMij = stg.tile([4, 10, E], dtm, tag="TMij")
            nc.sync.dma_start(out=TMij[0:2], in_=qt_rows(0, 20))
            nc.sync.dma_start(
                out=TMij[2:4],
                in_=bass.AP(tensor=rm_ij.tensor, offset=rm_ij.offset,
                            ap=[rm_ij.ap[0], [0, 10], [1, E]]))
            TMii = stg.tile([4, 5, E], dtm, tag="TMii")
            nc.sync.dma_start(out=TMii[0:2], in_=qt_rows(40, 50))
            nc.sync.dma_start(
                out=TMii[2:4],
                in_=bass.AP(tensor=rm_ii.tensor, offset=rm_ii.offset,
                            ap=[rm_ii.ap[0], [0, 5], [1, E]]))
            TIij = stg.tile([2, 10, E], dtm, tag="TIij")
            nc.sync.dma_start(out=TIij[0:2], in_=qt_rows(20, 40))
            TIii = stg.tile([2, 5, E], dtm, tag="TIii")
            nc.sync.dma_start(out=TIii[0:2], in_=qt_rows(50, 60))

            # ---------------- per-coefficient chains ----------------
            C = cp.tile([16, E], f32, tag="C")
            n_acc = [0]
            total_acc = 30

            def cacc(lhsT_tile, rhs_tile):
                k = n_acc[0]; n_acc[0] += 1
                mm(C, lhsT_tile, rhs_tile, start=(k == 0), stop=(k == total_acc - 1))

            for p in range(10):
                rhs = TMij[:, p]
                z1 = zp.tile([128, E], f32, tag="z")
                mm(z1, W["W1_Mij"], rhs)
                h1 = hp.tile([128, E], dtm, tag="h")
                evict(h1, z1)
                h3 = chain128(h1, "W2_Mij", "W3_Mij")
                cacc(W[f"W4C_Mij_{p}"], h3)
            for i in range(5):
                rhs = TMii[:, i]
                z1 = zp.tile([128, E], f32, tag="z")
                mm(z1, W["W1_Mii"], rhs)
                h1 = hp.tile([128, E], dtm, tag="h")
                evict(h1, z1)
                h3 = chain128(h1, "W2_Mii", "W3_Mii")
                cacc(W[f"W4C_Mii_{i}"], h3)

            if FI_MODE == "direct":
                for p in range(10):
                    rhs = TIij[:, p]
                    z1 = zp.tile([128, E], f32, tag="z")
                    mm(z1, W["W1_Iij"], rhs)
                    h1 = hp.tile([128, E], dtm, tag="h")
                    evict(h1, z1)
                    h3 = chain128(h1, "W2_Iij", "W3_Iij")
                    cacc(W["W4C_Iij"], h3)
                for i in range(5):
                    rhs = TIii[:, i]
                    z1 = zp.tile([128, E], f32, tag="z")
                    mm(z1, W["W1_Iii"], rhs)
                    h1 = hp.tile([128, E], dtm, tag="h")
                    evict(h1, z1)
                    h3 = chain128(h1, "W2_Iii", "W3_Iii")
                    cacc(W["W4C_Iii"], h3)
            else:
                for p in range(10):
                    rhs = TIij[:, p]
                    zo = zp.tile([128, E], f32, tag="z")
                    mm(zo, W["WO_Iij"], rhs)
                    U = hp.tile([128, E], dtm, tag="h")
                    evict(U, zo)
                    cacc(W["GAM_Iij"], U)
                for i in range(5):
                    rhs = TIii[:, i]
                    zo = zp.tile([128, E], f32, tag="z")
                    mm(zo, W["WO_Iii"], rhs)
                    U = hp.tile([128, E], dtm, tag="h")
                    evict(U, zo)
                    cacc(W["GAM_Iii"], U)

            # ---------------- finale ----------------
            C_sb = outp.tile([16, E], f32, tag="C_sb")
            nc.vector.tensor_copy(C_sb, C)
            C_e = outp.tile([128, NJ, 16], f32, tag="C_e")
            for j in range(NJ):
                Ct = zp.tile([128, E], f32, tag="z", name="Ct")[:, 0:16]
                nc.tensor.transpose(Ct, C_sb[:, j * 128:(j + 1) * 128], W["I16"])
                nc.vector.tensor_copy(C_e[:, j], Ct)

            prod = outp.tile([128, NJ, 144], f32, tag="prod")
            ce_b = bass.AP(tensor=C_e.tensor, offset=C_e.offset,
                           ap=[C_e.ap[0], [16, NJ], [1, 16], [0, 9]])
            nc.vector.tensor_tensor(
                prod, ce_b, xt.rearrange("p j (k n) -> p j k n", n=9), Alu.mult)
            ot = outp.tile([128, NJ, 9], f32, tag="ot")
            prod_v = bass.AP(tensor=prod.tensor, offset=prod.offset,
                             ap=[prod.ap[0], [144, NJ], [1, 9], [9, 16]])
            nc.vector.tensor_reduce(ot, prod_v, mybir.AxisListType.X, Alu.add)
            nc.sync.dma_start(
                out=bass.AP(tensor=d_out, offset=e0 * 9,
                            ap=[[9, 128], [9 * 128, NJ], [1, 9]]),
                in_=ot)

    _hoist_matmul_waits(nc)
    return nc


def _hoist_matmul_waits(nc):
    """Walrus codegen allows only one sync-wait on a (self-loading) matmul's
    LDWEIGHTS struct; hoist extra waits onto PE NoOps inserted just before."""
    import concourse.mybir as mybir
    import bass_rust
    k = 0
    for fn in nc.m.functions:
        for b in fn.blocks:
            new = []
            changed = False
            for inst in b.instructions:
                si = inst.sync_info
                if si is not None and len(si.on_wait) > 1:
                    waits = list(si.on_wait)
                    for w in waits[:-1]:
                        k += 1
                        nop = mybir.InstNoOp(name=f"I-whoist-{k}")
                        nop.engine = inst.engine
                        nop.sync_info = bass_rust.SyncInfo(on_wait=[w], on_update=[])
                        new.append(nop)
                    inst.sync_info = bass_rust.SyncInfo(
                        on_wait=[waits[-1]], on_update=list(si.on_update))
                    changed = True
                new.append(inst)
            if changed:
                b.instructions = new


# ======================================================================
# public entry point
# ======================================================================

TRACE = False          # set by test harness to collect exec_time_ns
LAST_RESULT = [None]


def kernel(scalars, x, params):
    import concourse.bass_utils as bass_utils

    scalars = np.ascontiguousarray(np.asarray(scalars, np.float32))
    x = np.ascontiguousarray(np.asarray(x, np.float32))
    B = scalars.shape[0]
    assert B % NCORES == 0
    bc = B // NCORES

    cst, g = _build_constants(params, scalars if FI_MODE == "refit" else None)
    blob = cst.blob()

    key = (bc, FP32R, FI_MODE, E_CHUNK)
    if key not in _COMPILE_CACHE:
        _COMPILE_CACHE[key] = _build_nc(bc, cst, g)
    nc = _COMPILE_CACHE[key]

    sc_sh = scalars.reshape(NCORES, bc, 45)
    x_sh = x.reshape(NCORES, bc, 144)
    in_maps = [{"scalars": np.ascontiguousarray(sc_sh[i]),
                "x": np.ascontiguousarray(x_sh[i]),
                "cst": blob} for i in range(NCORES)]
    res = bass_utils.run_bass_kernel_spmd(nc, in_maps, core_ids=list(range(NCORES)),
                                          trace=TRACE)
    LAST_RESULT[0] = res
    out = np.concatenate([r["out"] for r in res.results], axis=0)
    return out.astype(np.float32)


# revision 32
# speedup vs baseline: 2.3738x; 2.3738x over previous
"""Trainium2 Bass kernel for nn_EquivariancePermutationLayer.

Math restructuring (all MLPs have ZERO biases => positively homogeneous):
  * 1-D MLPs (f_Imij/f_Imii/f_Mmimj/f_Mmimi) collapse exactly to
        g(s) = g(1)*relu(s) + g(-1)*relu(-s)            (2 constants)
  * output = sum_{k=0..15} C[b,k] * x[b,k,:,:] with 16 per-element coeffs:
        C[:, 0:5]  = f_Mii(sii2_i, v4_i, rii, mii)       (direct MLP)
        C[:, 5:15] = f_Mij(sij2_p, v3_p, rij, mij)       (direct MLP)
        C[:, 15]   = sum_p f_Iij(sij2_p, v1_p) + sum_i f_Iii(sii2_i, v2_i)
  * globals rij/mij/rii/mii: direct MLPs on 15-D / 5-D inputs.
  * All per-(row) MLP layers run as TensorE matmuls with rows streamed on
    the free dim (hidden on partitions); the 16 coefficients accumulate
    into one [16, E] PSUM tile via column-packed final-layer stationaries.
  * optional 'refit' mode for the 2-D nets: exact homogenized PWL form
    F(u,v) = a+u+ + b+v+ + sum_k g+_k relu(v+ - t_k u+) + (mirror u<0),
    with least-squares-refit kink tables (approximation, ~2.5e-3 rel-l2).

Data parallel across 8 NeuronCores (16384 elements each), no collectives.
"""

import numpy as np

NCORES = 8
H = 128

# ---- tuning knobs ------------------------------------------------------
FP32R = True          # use float32r (fast reduced-precision) matmuls
FI_MODE = "direct"    # "direct" (exact) or "refit" (approx, fewer PE cols)
REFIT_K = 61          # kinks per half-plane in refit mode (61+61+6 = 128)
E_CHUNK = 512         # elements per chunk (multiple of 128; PSUM bank = 512 fp32)

_COMPILE_CACHE = {}


# ======================================================================
# host-side math helpers
# ======================================================================

def _np_params(params):
    return {k: [(np.asarray(W, np.float64), np.asarray(b, np.float64))
                for W, b in v] for k, v in params.items()}


def _mlp64(layers, h):
    for W, b in layers[:-1]:
        h = np.maximum(h @ W + b, 0)
    W, b = layers[-1]
    return h @ W + b


def _fit_homog_tables(layers, u, v, n_kinks):
    """Least-squares homogenized-PWL refit of a 2-input zero-bias MLP."""
    uf = u.ravel().astype(np.float64)
    vf = v.ravel().astype(np.float64)
    tabs = {}
    for side in (+1, -1):
        mask = (uf > 0) if side > 0 else (uf <= 0)
        us = np.abs(uf[mask])
        t = vf[mask] / np.maximum(us, 1e-300)
        w = us
        order = np.argsort(t)
        tw = t[order]
        cw = np.cumsum(w[order] ** 2)
        cw /= cw[-1]
        kinks = np.unique(np.interp(np.linspace(0.002, 0.998, n_kinks), cw, tw))
        ns = min(300_000, len(t))
        idx = np.random.default_rng(0).choice(len(t), ns, replace=False)
        ts, ws = t[idx], w[idx]
        phi = _mlp64(layers, np.stack([np.full_like(ts, side), ts], -1))[:, 0]
        A = np.concatenate(
            [np.ones((ns, 1)), ts[:, None], np.maximum(ts[:, None] - kinks[None, :], 0)], 1)
        coef, *_ = np.linalg.lstsq(A * ws[:, None], phi * ws, rcond=None)
        # pad kink count up to n_kinks (duplicates with zero gamma)
        kk = np.zeros(n_kinks); gg = np.zeros(n_kinks)
        kk[:len(kinks)] = kinks
        kk[len(kinks):] = kinks[-1] if len(kinks) else 0.0
        gg[:len(kinks)] = coef[2:]
        tabs[side] = (np.float64(coef[0]), np.float64(coef[1]), kk, gg)
    return tabs


# ======================================================================
# constants blob layout
# ======================================================================

class CstPacker:
    def __init__(self):
        self.chunks = []
        self.offsets = {}
        self.size = 0

    def add(self, name, arr):
        arr = np.ascontiguousarray(np.asarray(arr, np.float32))
        self.offsets[name] = (self.size, arr.shape)
        self.chunks.append(arr.ravel())
        self.size += arr.size

    def blob(self):
        return np.concatenate(self.chunks).astype(np.float32)


def _build_constants(params, scalars=None):
    """Pack all device constants. scalars (full array) only needed for refit."""
    P = _np_params(params)
    cst = CstPacker()

    # g-collapse constants
    g = {}
    for n in ["f_Imij", "f_Imii", "f_Mmimj", "f_Mmimi"]:
        cp = _mlp64(P[n], np.array([[1.0]]))[0, 0]
        cn = _mlp64(P[n], np.array([[-1.0]]))[0, 0]
        g[n] = (np.float32(cp), np.float32(cn))

    def addW(name, key, li):
        cst.add(name, P[key][li][0])

    # global nets (full weights)
    for key, tag in [("f_Mrij", "rij"), ("f_Mmij", "mij"),
                     ("f_Mrii", "rii"), ("f_Mmii", "mii")]:
        addW(f"W1_{tag}", key, 0)
        addW(f"W2_{tag}", key, 1)
        addW(f"W3_{tag}", key, 2)
    # packed final layer of globals -> RM psum rows (rij, mij, rii, mii)
    for j, key in enumerate(["f_Mrij", "f_Mmij", "f_Mrii", "f_Mmii"]):
        w4 = np.zeros((128, 4), np.float32)
        w4[:, j] = P[key][3][0][:, 0]
        cst.add(f"W4RM_{j}", w4)

    # F_M nets
    for key, tag in [("f_Mij", "Mij"), ("f_Mii", "Mii")]:
        addW(f"W1_{tag}", key, 0)     # [4,128]
        addW(f"W2_{tag}", key, 1)
        addW(f"W3_{tag}", key, 2)
    for p in range(10):
        w4 = np.zeros((128, 16), np.float32)
        w4[:, 5 + p] = P["f_Mij"][3][0][:, 0]
        cst.add(f"W4C_Mij_{p}", w4)
    for i in range(5):
        w4 = np.zeros((128, 16), np.float32)
        w4[:, i] = P["f_Mii"][3][0][:, 0]
        cst.add(f"W4C_Mii_{i}", w4)

    # F_I nets
    if FI_MODE == "direct":
        for key, tag in [("f_Iij", "Iij"), ("f_Iii", "Iii")]:
            addW(f"W1_{tag}", key, 0)   # [2,128]
            addW(f"W2_{tag}", key, 1)
            addW(f"W3_{tag}", key, 2)
            w4 = np.zeros((128, 16), np.float32)
            w4[:, 15] = P[key][3][0][:, 0]
            cst.add(f"W4C_{tag}", w4)
    else:
        assert scalars is not None
        sc = np.asarray(scalars, np.float32)
        sii = sc[:, :5]; sij = sc[:, 5:]
        s0, s1 = sij[..., 0], sij[..., 1]
        v1 = g["f_Imij"][0] * (np.maximum(s0, 0) + np.maximum(s1, 0)) \
           + g["f_Imij"][1] * (np.maximum(-s0, 0) + np.maximum(-s1, 0))
        d0 = sii[..., 0]
        v2 = g["f_Imii"][0] * np.maximum(d0, 0) + g["f_Imii"][1] * np.maximum(-d0, 0)
        for key, tag, uu, vv in [("f_Iij", "Iij", sij[..., 2], v1),
                                 ("f_Iii", "Iii", sii[..., 2], v2)]:
            tb = _fit_homog_tables(P[key], uu, vv, REFIT_K)
            # outer stationary [4, 128]: rows (u+, v+, u-, v-);
            # cols 0..K-1:   (-t+_k, 1, 0, 0)
            # cols K..2K-1:  (0, 0, -t-_k, 1)
            # cols 2K..2K+5: u+ | v+ | -v+ | u- | v- | -v-
            K = REFIT_K
            ap, bp, tp, gp = tb[+1]
            an, bn, tn, gn = tb[-1]
            W = np.zeros((4, 128), np.float32)
            W[0, :K] = -tp; W[1, :K] = 1.0
            W[2, K:2*K] = -tn; W[3, K:2*K] = 1.0
            W[0, 2*K] = 1.0
            W[1, 2*K+1] = 1.0; W[1, 2*K+2] = -1.0
            W[2, 2*K+3] = 1.0
            W[3, 2*K+4] = 1.0; W[3, 2*K+5] = -1.0
            cst.add(f"WO_{tag}", W)
            gam = np.zeros((128, 16), np.float32)
            gam[:K, 15] = gp
            gam[K:2*K, 15] = gn
            gam[2*K:2*K+6, 15] = [ap, bp, bp, an, bn, bn]
            gam[2*K+2, 15] *= -1.0   # -b+ relu(-v+)
            gam[2*K+5, 15] *= -1.0
            cst.add(f"GAM_{tag}", gam)

    cst.add("I16", np.eye(16, dtype=np.float32))
    cst.add("I128", np.eye(128, dtype=np.float32))
    return cst, g


# ======================================================================
# device kernel builder
# ======================================================================

def _build_nc(n_elem, cst_meta, g_const):
    import concourse.bass as bass
    import concourse.mybir as mybir
    from concourse.tile import TileContext
    from concourse.tile_rust import add_dep_helper
    from contextlib import ExitStack

    f32 = mybir.dt.float32
    f32r = mybir.dt.float32r
    Relu = mybir.ActivationFunctionType.Relu
    Alu = mybir.AluOpType

    E = E_CHUNK
    NJ = E // 128                     # e-groups per chunk
    NCH = n_elem // E
    assert n_elem % E == 0

    nc = bass.Bass()
    d_sc = nc.declare_dram_parameter("scalars", (n_elem, 45), f32, isOutput=False)
    d_x = nc.declare_dram_parameter("x", (n_elem, 144), f32, isOutput=False)
    d_cst = nc.declare_dram_parameter("cst", (cst_meta.size,), f32, isOutput=False)
    d_out = nc.declare_dram_parameter("out", (n_elem, 9), f32, isOutput=True)

    (c1p, c1n) = map(float, g_const["f_Imij"])
    (c2p, c2n) = map(float, g_const["f_Imii"])
    (c3p, c3n) = map(float, g_const["f_Mmimj"])
    (c4p, c4n) = map(float, g_const["f_Mmimi"])

    dtm = f32r if FP32R else f32

    def r_(ap):
        return ap

    with TileContext(nc) as tc, ExitStack() as ctx:
        consts = ctx.enter_context(tc.tile_pool(name="consts", bufs=1))
        iop = ctx.enter_context(tc.tile_pool(name="iop", bufs=3))
        elp = ctx.enter_context(tc.tile_pool(name="elp", bufs=3))
        stg = ctx.enter_context(tc.tile_pool(name="stg", bufs=2))
        hp = ctx.enter_context(tc.tile_pool(name="hp", bufs=8))
        outp = ctx.enter_context(tc.tile_pool(name="outp", bufs=4))
        zp = ctx.enter_context(tc.tile_pool(name="zp", bufs=6, space="PSUM"))
        cp = ctx.enter_context(tc.tile_pool(name="cp", bufs=2, space="PSUM"))

        ev_ctr = [0]

        def evict(out_ap, in_ap, relu=True, eng=None):
            if eng is None:
                eng = "v" if (ev_ctr[0] * 5) % 11 < 5 else "a"
                ev_ctr[0] += 1
            if eng == "v":
                if relu:
                    return nc.vector.tensor_scalar_max(out_ap, in_ap, 0.0)
                return nc.vector.tensor_copy(out_ap, in_ap)
            return nc.scalar.activation(
                out_ap, in_ap, Relu if relu else mybir.ActivationFunctionType.Copy)

        def _raw(i):
            return i.ins if hasattr(i, "ins") and not isinstance(i.ins, list) else i

        def mm(out, lhsT, rhs, start=True, stop=True, after=None):
            return nc.tensor.matmul(out, r_(lhsT), r_(rhs), start=start, stop=stop)

        def chain128(h1, tagW2, tagW3):
            """h1 [128,E] sbuf -> returns h3 [128,E] sbuf"""
            z2 = zp.tile([128, E], f32, tag="z")
            mm(z2, W[tagW2], h1)
            h2 = hp.tile([128, E], dtm, tag="h")
            evict(h2, z2)
            z3 = zp.tile([128, E], f32, tag="z")
            mm(z3, W[tagW3], h2)
            h3 = hp.tile([128, E], dtm, tag="h")
            evict(h3, z3)
            return h3

        # ---- load constants once ----
        W = {}
        for name, (off, shape) in cst_meta.offsets.items():
            dt_w = f32 if name in ("I16", "I128") else dtm
            t = consts.tile(list(shape), dt_w, tag=f"c_{name}")
            src = bass.AP(tensor=d_cst, offset=off,
                          ap=[[shape[-1], shape[0]], [1, shape[-1]]]
                          if len(shape) == 2 else [[1, shape[0]]])
            if dt_w is not f32:
                src = src.bitcast(dt_w)
            nc.sync.dma_start(out=t, in_=src)
            W[name] = t

        for ch in range(NCH):
            e0 = ch * E

            # ---------------- loads ----------------
            sraw = iop.tile([128, NJ, 45], f32, tag="sraw")
            nc.sync.dma_start(
                out=sraw,
                in_=bass.AP(tensor=d_sc, offset=e0 * 45,
                            ap=[[45, 128], [45 * 128, NJ], [1, 45]]))
            xt = iop.tile([128, NJ, 144], f32, tag="xt")
            nc.sync.dma_start(
                out=xt,
                in_=bass.AP(tensor=d_x, offset=e0 * 144,
                            ap=[[144, 128], [144 * 128, NJ], [1, 144]]))

            # ---------------- e-layout g computation ----------------
            # Q columns: 0..9 uA | 10..19 v3 | 20..29 uB | 30..39 v1
            #   40..44 udA | 45..49 v4 | 50..54 udB | 55..59 v2
            #   60..74 sbar | 75..79 sii0
            QC = 80
            Q = elp.tile([128, NJ, QC], f32, tag="Q")

            def sl(col0, stride, n):
                return sraw[:, :, col0:col0 + stride * (n - 1) + 1:stride]

            s0ap = sl(15, 3, 10); s1ap = sl(16, 3, 10); d0ap = sl(0, 3, 5)

            # raw input columns
            nc.vector.tensor_copy(Q[:, :, 0:10], sl(17, 3, 10))
            nc.vector.tensor_copy(Q[:, :, 20:30], sl(17, 3, 10))
            nc.vector.tensor_copy(Q[:, :, 40:45], sl(2, 3, 5))
            nc.vector.tensor_copy(Q[:, :, 50:55], sl(2, 3, 5))
            nc.vector.tensor_copy(Q[:, :, 60:75], sl(2, 3, 15))
            nc.vector.tensor_copy(Q[:, :, 75:80], sl(0, 3, 5))

            t0 = elp.tile([128, NJ, 10], f32, tag="t0")
            t1 = elp.tile([128, NJ, 10], f32, tag="t1")
            Sp = elp.tile([128, NJ, 10], f32, tag="Sp")
            nc.vector.tensor_scalar_max(t0, s0ap, 0.0)
            nc.vector.tensor_scalar_max(t1, s1ap, 0.0)
            nc.vector.tensor_tensor(Sp, t0, t1, Alu.add)
            t0n = elp.tile([128, NJ, 10], f32, tag="t0n")
            t1n = elp.tile([128, NJ, 10], f32, tag="t1n")
            Sn = elp.tile([128, NJ, 10], f32, tag="Sn")
            nc.vector.tensor_scalar(t0n, s0ap, -1.0, 0.0, Alu.mult, Alu.max)
            nc.vector.tensor_scalar(t1n, s1ap, -1.0, 0.0, Alu.mult, Alu.max)
            nc.vector.tensor_tensor(Sn, t0n, t1n, Alu.add)

            tmp = elp.tile([128, NJ, 10], f32, tag="tmpg")
            nc.vector.tensor_scalar_mul(tmp, Sp, c3p)
            nc.vector.scalar_tensor_tensor(Q[:, :, 10:20], Sn, c3n, tmp, Alu.mult, Alu.add)
            nc.vector.tensor_scalar_mul(tmp, Sp, c1p)
            nc.vector.scalar_tensor_tensor(Q[:, :, 30:40], Sn, c1n, tmp, Alu.mult, Alu.add)

            dp = elp.tile([128, NJ, 5], f32, tag="dp")
            dn = elp.tile([128, NJ, 5], f32, tag="dn")
            nc.vector.tensor_scalar_max(dp, d0ap, 0.0)
            nc.vector.tensor_scalar(dn, d0ap, -1.0, 0.0, Alu.mult, Alu.max)
            tmpd = elp.tile([128, NJ, 5], f32, tag="tmpd")
            nc.vector.tensor_scalar_mul(tmpd, dp, c4p)
            nc.vector.scalar_tensor_tensor(Q[:, :, 45:50], dn, c4n, tmpd, Alu.mult, Alu.add)
            nc.vector.tensor_scalar_mul(tmpd, dp, c2p)
            nc.vector.scalar_tensor_tensor(Q[:, :, 55:60], dn, c2n, tmpd, Alu.mult, Alu.add)

            # ---------------- transpose Q -> QT (features on partitions) ----
            QT = elp.tile([QC, NJ, 128], dtm, tag="QT")
            for j in range(NJ):
                zt = zp.tile([128, E], f32, tag="z", name="zt")
                nc.tensor.transpose(zt[0:QC, 0:128], Q[:, j, :], W["I128"])
                evict(QT[:, j, :], zt[0:QC, 0:128], relu=False)

            # ---------------- globals (PE) ----------------
            def qt_rows(a, b):
                return QT[a:b].rearrange("q j i -> q (j i)")

            Tg15 = stg.tile([15, E], dtm, tag="Tg15")
            nc.sync.dma_start(out=Tg15, in_=qt_rows(60, 75))
            Tg5 = stg.tile([5, E], dtm, tag="Tg5")
            nc.sync.dma_start(out=Tg5, in_=qt_rows(75, 80))

            def run_chain_group(specs, final_fn):
                """specs: list of (rhs, w1, w2, w3, tag); layer-major."""
                zs = []
                for (rhs, w1, w2, w3, w4c) in specs:
                    z = zp.tile([128, E], f32, tag="z", name="z1")
                    mm(z, W[w1], rhs)
                    zs.append(z)
                h1s = []
                for z in zs:
                    h = hp.tile([128, E], dtm, tag="h", name="h1")
                    evict(h, z)
                    h1s.append(h)
                z2s = []
                for h, (rhs, w1, w2, w3, w4c) in zip(h1s, specs):
                    z = zp.tile([128, E], f32, tag="z", name="z2")
                    mm(z, W[w2], h)
                    z2s.append(z)
                h2s = []
                for z in z2s:
                    h = hp.tile([128, E], dtm, tag="h", name="h2")
                    evict(h, z)
                    h2s.append(h)
                z3s = []
                for h, (rhs, w1, w2, w3, w4c) in zip(h2s, specs):
                    z = zp.tile([128, E], f32, tag="z", name="z3")
                    mm(z, W[w3], h)
                    z3s.append(z)
                h3s = []
                for z in z3s:
                    h = hp.tile([128, E], dtm, tag="h", name="h3")
                    evict(h, z)
                    h3s.append(h)
                for k, (h, (rhs, w1, w2, w3, w4c)) in enumerate(zip(h3s, specs)):
                    final_fn(k, w4c, h)

            RM = cp.tile([4, E], f32, tag="cps", name="RM")
            gspecs = [(Tg15 if t in ("rij", "rii") else Tg5,
                       f"W1_{t}", f"W2_{t}", f"W3_{t}", f"W4RM_{j}")
                      for j, t in enumerate(["rij", "mij", "rii", "mii"])]
            run_chain_group(
                gspecs,
                lambda k, w4c, h: mm(RM, W[w4c], h, start=(k == 0), stop=(k == 3)))
            rm4 = outp.tile([4, E], dtm, tag="rm4")
            nc.vector.tensor_copy(rm4, RM)
            rm_ij = rm4[0:2]
            rm_ii = rm4[2:4]

            # ---------------- staging tiles [K, P, E] ----------------
            TMij = stg.tile([4, 10, E], dtm, tag="TMij")
            nc.sync.dma_start(out=TMij[0:2], in_=qt_rows(0, 20))
            nc.sync.dma_start(
                out=TMij[2:4],
                in_=bass.AP(tensor=rm_ij.tensor, offset=rm_ij.offset,
                            ap=[rm_ij.ap[0], [0, 10], [1, E]]))
            TMii = stg.tile([4, 5, E], dtm, tag="TMii")
            nc.sync.dma_start(out=TMii[0:2], in_=qt_rows(40, 50))
            nc.sync.dma_start(
                out=TMii[2:4],
                in_=bass.AP(tensor=rm_ii.tensor, offset=rm_ii.offset,
                            ap=[rm_ii.ap[0], [0, 5], [1, E]]))
            TIij = stg.tile([2, 10, E], dtm, tag="TIij")
            nc.sync.dma_start(out=TIij[0:2], in_=qt_rows(20, 40))
            TIii = stg.tile([2, 5, E], dtm, tag="TIii")
            nc.sync.dma_start(out=TIii[0:2], in_=qt_rows(50, 60))

            # ---------------- per-coefficient chains ----------------
            C = cp.tile([16, E], f32, tag="cps", name="C")
            n_acc = [0]
            total_acc = 30

            def cacc(lhsT_tile, rhs_tile):
                k = n_acc[0]; n_acc[0] += 1
                mm(C, lhsT_tile, rhs_tile, start=(k == 0), stop=(k == total_acc - 1))

            specs = [(TMij[:, p], "W1_Mij", "W2_Mij", "W3_Mij", f"W4C_Mij_{p}")
                     for p in range(10)]
            specs += [(TMii[:, i], "W1_Mii", "W2_Mii", "W3_Mii", f"W4C_Mii_{i}")
                      for i in range(5)]
            if FI_MODE == "direct":
                specs += [(TIij[:, p], "W1_Iij", "W2_Iij", "W3_Iij", "W4C_Iij")
                          for p in range(10)]
                specs += [(TIii[:, i], "W1_Iii", "W2_Iii", "W3_Iii", "W4C_Iii")
                          for i in range(5)]
            G = 6
            for a in range(0, len(specs), G):
                run_chain_group(specs[a:a + G],
                                lambda k, w4c, h: cacc(W[w4c], h))

            if FI_MODE == "refit":
                for p in range(10):
                    rhs = TIij[:, p]
                    zo = zp.tile([128, E], f32, tag="z")
                    mm(zo, W["WO_Iij"], rhs)
                    U = hp.tile([128, E], dtm, tag="h")
                    evict(U, zo)
                    cacc(W["GAM_Iij"], U)
                for i in range(5):
                    rhs = TIii[:, i]
                    zo = zp.tile([128, E], f32, tag="z")
                    mm(zo, W["WO_Iii"], rhs)
                    U = hp.tile([128, E], dtm, tag="h")
                    evict(U, zo)
                    cacc(W["GAM_Iii"], U)

            # ---------------- finale ----------------
            C_sb = outp.tile([16, E], f32, tag="C_sb")
            nc.vector.tensor_copy(C_sb, C)
            C_e = outp.tile([128, NJ, 16], f32, tag="C_e")
            for j in range(NJ):
                Ct = zp.tile([128, E], f32, tag="z", name="Ct")[:, 0:16]
                nc.tensor.transpose(Ct, C_sb[:, j * 128:(j + 1) * 128], W["I16"])
                nc.vector.tensor_copy(C_e[:, j], Ct)

            prod = outp.tile([128, NJ, 144], f32, tag="prod")
            ce_b = bass.AP(tensor=C_e.tensor, offset=C_e.offset,
                           ap=[C_e.ap[0], [16, NJ], [1, 16], [0, 9]])
            nc.vector.tensor_tensor(
                prod, ce_b, xt.rearrange("p j (k n) -> p j k n", n=9), Alu.mult)
            ot = outp.tile([128, NJ, 9], f32, tag="ot")
            prod_v = bass.AP(tensor=prod.tensor, offset=prod.offset,
                             ap=[prod.ap[0], [144, NJ], [1, 9], [9, 16]])
            nc.vector.tensor_reduce(ot, prod_v, mybir.AxisListType.X, Alu.add)
            nc.sync.dma_start(
                out=bass.AP(tensor=d_out, offset=e0 * 9,
                            ap=[[9, 128], [9 * 128, NJ], [1, 9]]),
                in_=ot)

    _hoist_matmul_waits(nc)
    return nc


def _hoist_matmul_waits(nc):
    """Walrus codegen allows only one sync-wait on a (self-loading) matmul's
    LDWEIGHTS struct; hoist extra waits onto PE NoOps inserted just before."""
    import concourse.mybir as mybir
    import bass_rust
    k = 0
    for fn in nc.m.functions:
        for b in fn.blocks:
            new = []
            changed = False
            for inst in b.instructions:
                si = inst.sync_info
                if si is not None and len(si.on_wait) > 1:
                    waits = list(si.on_wait)
                    for w in waits[:-1]:
                        k += 1
                        nop = mybir.InstNoOp(name=f"I-whoist-{k}")
                        nop.engine = inst.engine
                        nop.sync_info = bass_rust.SyncInfo(on_wait=[w], on_update=[])
                        new.append(nop)
                    inst.sync_info = bass_rust.SyncInfo(
                        on_wait=[waits[-1]], on_update=list(si.on_update))
                    changed = True
                new.append(inst)
            if changed:
                b.instructions = new


# ======================================================================
# public entry point
# ======================================================================

TRACE = False          # set by test harness to collect exec_time_ns
LAST_RESULT = [None]


def kernel(scalars, x, params):
    import concourse.bass_utils as bass_utils

    scalars = np.ascontiguousarray(np.asarray(scalars, np.float32))
    x = np.ascontiguousarray(np.asarray(x, np.float32))
    B = scalars.shape[0]
    assert B % NCORES == 0
    bc = B // NCORES

    cst, g = _build_constants(params, scalars if FI_MODE == "refit" else None)
    blob = cst.blob()

    key = (bc, FP32R, FI_MODE, E_CHUNK)
    if key not in _COMPILE_CACHE:
        _COMPILE_CACHE[key] = _build_nc(bc, cst, g)
    nc = _COMPILE_CACHE[key]

    sc_sh = scalars.reshape(NCORES, bc, 45)
    x_sh = x.reshape(NCORES, bc, 144)
    in_maps = [{"scalars": np.ascontiguousarray(sc_sh[i]),
                "x": np.ascontiguousarray(x_sh[i]),
                "cst": blob} for i in range(NCORES)]
    res = bass_utils.run_bass_kernel_spmd(nc, in_maps, core_ids=list(range(NCORES)),
                                          trace=TRACE)
    LAST_RESULT[0] = res
    out = np.concatenate([r["out"] for r in res.results], axis=0)
    return out.astype(np.float32)
